# revision 1
# baseline (speedup 1.0000x reference)
"""Chamfer loss kernel for Trainium2 (8 NeuronCores).

Problem: pred [4,8192,3], gt [4,8192,3] ->
  mean_b( mean_n min_m ||p_bn - g_bm||^2 + mean_m min_n ||p_bn - g_bm||^2 )

Sharding: 8 shards = (batch b in 0..3) x (half of N). Each core gets
pred half [4096,3] + full gt [8192,3] of its batch and computes
  - rowmins: min over all m for each of its 4096 pred rows
  - colpart: min over its 4096 pred rows for each of the 8192 gt points
Host combines (concat rows / min cols), means, final scalar.

Device algorithm (per core): distances via a single K=5 matmul using
augmented vectors  paug=[p0,p1,p2,|p|^2,1], gaug=[2g0,2g1,2g2,-1,-|g|^2]
so  paug . gaug = -(|p|^2+|g|^2-2p.g) = -d  (negated -> all reductions are max).
Loop j over 16 gt chunks of 512, i over 32 pred chunks of 128:
  PE:  psum[128,512] = -d block
  ACT: copy/cast psum -> fp16 chunk of a grouped SBUF tile
  DVE: tensor_scalar(+0, accum=max) -> row max per (i,j) chunk
  DVE: tensor_tensor max into gmax_j (running col max over i)
Finalize each j: DMA-transpose gmax_j 128-blocks + DVE max-reduce
-> per-gt-point col max. Final: reduce row accs, DMA out [128, 96].

walrus quirk: every TPB compute instruction can carry at most ONE sync
wait.  Two measures keep us legal: (a) fp16 chunks live in two ping-pong
group tiles [128, G, 512]; before writing a group, one tiny ACT "spacer"
op overwrites column 0 of every chunk, absorbing the WAR-on-DVE wait so
each real copy carries only its PE wait; (b) Tile's redundant
same-engine ACT self-waits are stripped post-trace.
"""

import numpy as np

import concourse.bass as bass
import concourse.mybir as mybir
import concourse.tile as tile
from concourse.bass_utils import run_bass_kernel_spmd

B, N, M = 4, 8192, 8192
NCORES = 8
NSH = N // 2  # pred rows per core
P = 128  # partition tile (pred rows per matmul)
FD = 512  # matmul free dim (gt cols per matmul)
NI = NSH // P  # 32
NJ = M // FD  # 16
G = 8  # chunks per ping-pong group tile
NEG_INF16 = -60000.0

_f32 = mybir.dt.float32
_f16 = mybir.dt.float16

_cache = {}


def _build_nc():
    nc = bass.Bass()
    # single fused input: [5, NSH pred-aug | M gt-aug] so the first matmul
    # waits on one DMA semaphore (LDWEIGHTS encodes only one sync wait)
    aT = nc.declare_dram_parameter("aT", [5, NSH + M], _f32, isOutput=False)
    # single output: cols 0:8192 = colmax partial (rows 0:16 valid, f16);
    # cols 8192:8256 = rowmins [128,32] f32 bitcast as f16 pairs
    outall = nc.declare_dram_parameter("outall", [P, NJ * FD + 64], _f16, isOutput=True)

    Alu = mybir.AluOpType
    with tile.TileContext(nc) as tc:
        with (
            tc.tile_pool(name="const", bufs=1) as cpool,
            tc.tile_pool(name="grp", bufs=2) as grp_pool,
            tc.tile_pool(name="acc", bufs=1) as apool,
            tc.tile_pool(name="psd", bufs=8, space="PSUM") as psumd,
        ):
            aTs = cpool.tile([5, NSH + M], _f32, tag="aT")
            nc.sync.dma_start(aTs[:], aT[:])
            paTs = aTs[:, :NSH]
            gaTs = aTs[:, NSH:]

            # rowacc[:, i*NJ+j] = max over chunk (i,j); reduced over j at the end
            rowacc = apool.tile([P, NI * NJ], _f32, tag="rowacc")
            junk = apool.tile([P, FD], _f16, tag="junk")
            # one big result tile: per-j col maxes side by side + rowmins tail
            big = apool.tile([P, NJ * FD + 64], _f16, tag="big")
            gmaxall = big[:, : NJ * FD]
            rowmins = big[:, NJ * FD :].bitcast(_f32)  # [128, 32] f32 view
            scratch64 = apool.tile([64, NJ * FD], _f16, tag="scratch64")
            scratch32 = apool.tile([32, NJ * FD], _f16, tag="scratch32")
            scratch16 = apool.tile([16, NJ * FD], _f16, tag="scratch16")
            scratch = {64: scratch64, 32: scratch32, 16: scratch16}

            dh_grp = None
            for j in range(NJ):
                gmax = gmaxall[:, j * FD : (j + 1) * FD]
                nc.vector.memset(gmax, NEG_INF16)
                for i in range(NI):
                    g = i % G
                    pt = psumd.tile([P, FD], _f32, tag="d")
                    nc.tensor.matmul(
                        pt[:],
                        paTs[:, i * P : (i + 1) * P],
                        gaTs[:, j * FD : (j + 1) * FD],
                        start=True,
                        stop=True,
                    )
                    if g == 0:
                        dh_grp = grp_pool.tile([P, G, FD], _f16, tag="dh")
                        # spacer: overwrite col 0 of every chunk; absorbs the
                        # WAR wait against last round's DVE readers so the
                        # real copies below carry only their PE wait
                        nc.scalar.mul(dh_grp[:, :, 0:1], dh_grp[:, :, 0:1], 0.0)
                    dh = dh_grp[:, g, :]
                    nc.scalar.copy(dh, pt[:])
                    c = i * NJ + j
                    nc.vector.tensor_scalar(
                        out=junk[:],
                        in0=dh,
                        scalar1=0.0,
                        scalar2=None,
                        op0=Alu.add,
                        op1=Alu.max,
                        accum_out=rowacc[:, c : c + 1],
                    )
                    nc.vector.tensor_tensor(
                        out=gmax, in0=gmax, in1=dh, op=Alu.max
                    )

            # fold partitions 128 -> 16 with SWDGE copies + DVE maxes
            for pk in (64, 32, 16):
                scr = scratch[pk]
                nc.gpsimd.dma_start(scr[:], gmaxall[pk : 2 * pk, :])
                nc.vector.tensor_tensor(
                    out=gmaxall[0:pk, :],
                    in0=gmaxall[0:pk, :],
                    in1=scr[:],
                    op=Alu.max,
                )
            nc.vector.tensor_reduce(
                out=rowmins,
                in_=rowacc[:].rearrange("p (i j) -> p i j", j=NJ),
                axis=mybir.AxisListType.X,
                op=Alu.max,
            )
            # single output DMA: its wait on DVE transitively covers every
            # engine, so the tail drain only needs this DMA's queue sem
            nc.gpsimd.dma_start(outall[:], big[:])

    _strip_self_waits(nc)
    _slim_drain(nc)
    return nc


def _slim_drain(nc):
    """Reduce the kernel-tail drain to one wait (walrus 1-wait limit).

    The final output DMA waits on DVE, whose tick transitively covers all
    compute engines and the tree-copy queues (each tree copy is awaited by
    a DVE max; every ACT/PE op funnels into DVE consumers; the aT load is
    awaited by the first matmul).  So the drain only needs the output
    DMA's own queue semaphore.
    """
    last_q = None
    for f in nc.m.functions:
        for blk in f.blocks:
            for ins in blk.instructions:
                if type(ins).__name__ == "InstDMACopy":
                    si = ins.sync_info
                    for u in si.on_update:
                        if u.ant_name.startswith("DMASW"):
                            last_q = u.ant_name
    assert last_q is not None
    for f in nc.m.functions:
        for blk in f.blocks:
            for ins in blk.instructions:
                if type(ins).__name__ != "InstDrain":
                    continue
                si = ins.sync_info
                if si is None or len(si.on_wait) <= 1:
                    continue
                keep = [w for w in si.on_wait if w.ant_name == last_q]
                assert keep, f"drain lost its output-queue wait: {si}"
                ins.sync_info = mybir.SyncInfo(
                    on_wait=keep, on_update=list(si.on_update)
                )


_ENGINE_SEM_PREFIX = {
    mybir.EngineType.Activation: "Activation",
    mybir.EngineType.DVE: "DVE",
    mybir.EngineType.PE: "PE",
    mybir.EngineType.Pool: "Pool",
    mybir.EngineType.SP: "SP",
}


def _strip_self_waits(nc):
    """Drop a compute instruction's waits on its own engine semaphore.

    Tile models the sequencer separately from the engine and emits
    same-engine waits for buffer-slot WAW/WAR reuse; the engines complete
    in order so these are redundant, and walrus's TPB structs only encode
    one sync wait (the cross-engine wait is the essential one).
    """
    for f in nc.m.functions:
        for blk in f.blocks:
            for ins in blk.instructions:
                eng = getattr(ins, "engine", None)
                pfx = _ENGINE_SEM_PREFIX.get(eng)
                if pfx is None or type(ins).__name__ == "InstDrain":
                    continue
                si = ins.sync_info
                if si is None or not si.on_wait:
                    continue
                w2 = [w for w in si.on_wait if not w.ant_name.startswith(pfx)]
                if len(w2) != len(si.on_wait):
                    ins.sync_info = mybir.SyncInfo(
                        on_wait=w2, on_update=list(si.on_update)
                    )


def _max_tpb_waits(nc):
    """(debug) max on_wait count over TPB compute instructions."""
    worst = (0, None)
    skip = {"InstDrain", "InstEventSemaphore", "InstISA", "InstRegisterMove"}
    for f in nc.m.functions:
        for blk in f.blocks:
            for ins in blk.instructions:
                t = type(ins).__name__
                if t in skip or t.startswith("InstDma"):
                    continue
                si = ins.sync_info
                nw = len(si.on_wait) if si else 0
                if nw > worst[0]:
                    worst = (nw, (ins.name, t, [w.ant_name for w in si.on_wait]))
    return worst


def _get_nc():
    if "nc" not in _cache:
        _cache["nc"] = _build_nc()
    return _cache["nc"]


def _augment(pred_h, gt_b):
    """pred_h [NSH,3], gt_b [M,3] -> aT [5, NSH+M] with
    aT[:,n] . aT[:,NSH+m] = -(squared distance n,m)."""
    aT = np.empty((5, NSH + M), np.float32)
    aT[0:3, :NSH] = pred_h.T
    aT[3, :NSH] = (pred_h * pred_h).sum(1)
    aT[4, :NSH] = 1.0
    aT[0:3, NSH:] = 2.0 * gt_b.T
    aT[3, NSH:] = -1.0
    aT[4, NSH:] = -(gt_b * gt_b).sum(1)
    return aT


def _run(pred, gt, **kwargs):
    nc = _get_nc()
    in_maps = []
    for c in range(NCORES):
        b, h = divmod(c, 2)
        in_maps.append({"aT": _augment(pred[b, h * NSH : (h + 1) * NSH], gt[b])})
    return run_bass_kernel_spmd(nc, in_maps, list(range(NCORES)), **kwargs)


def _split_out(r):
    o = r["outall"]
    colpart = o[0:16, : NJ * FD].astype(np.float32).max(axis=0)  # [M]
    rowm = np.ascontiguousarray(o[:, NJ * FD :]).view(np.float32)  # [128, NI]
    return colpart, rowm


def _combine(results):
    """results: list of 8 {'outall': [128, M+64] f16} -> scalar loss."""
    total = 0.0
    for b in range(B):
        c0, rm0 = _split_out(results[2 * b])
        c1, rm1 = _split_out(results[2 * b + 1])
        # rowmins[p, i] is pred row i*128+p -> transpose+flatten = shard order
        rm = np.concatenate([-rm0.T.reshape(-1), -rm1.T.reshape(-1)])
        cm = -np.maximum(c0, c1)
        total += rm.mean() + cm.mean()
    return np.float32(total / B)


def kernel(pred, gt):
    pred = np.ascontiguousarray(np.asarray(pred, dtype=np.float32))
    gt = np.ascontiguousarray(np.asarray(gt, dtype=np.float32))
    res = _run(pred, gt)
    return _combine(res.results)



# revision 3
# speedup vs baseline: 2.3895x; 2.3895x over previous
"""Chamfer loss kernel for Trainium2 (8 NeuronCores) — v2.

Problem: pred [4,8192,3], gt [4,8192,3] ->
  mean_b( mean_n min_m ||p_bn - g_bm||^2 + mean_m min_n ||p_bn - g_bm||^2 )

Sharding: 8 shards = (batch b in 0..3) x (half of N). Each core gets
pred half [4096,3] + full gt [8192,3] of its batch and computes
  - rowmins: min over all m for each of its 4096 pred rows
  - colpart: min over its 4096 pred rows for each of the 8192 gt points
    (as a [128, 8192] partition-partial; host folds 128->1)
Host combines (concat rows / min cols), means, final scalar.

v2 changes vs v1 (931us):
  * fp32 matmul (2 HW passes, PE-bound at 94%) replaced by a single
    fp16 matmul with K=15: each augmented vector is split hi/lo
    (hi=fp16(x), lo=fp16(x-hi)) and the three cross products
    Phi.Ghi + Phi.Glo + Plo.Ghi are stacked along the contraction dim:
    lhsT=[Phi;Phi;Plo], rhs=[Ghi;Glo;Ghi]. The dropped Plo.Glo term is
    O(1e-6) abs. PSUM accumulates fp32, so the result is fp32-accurate
    (verified 3.6e-5 rel on the full loss).
    aug: paug=[p,|p|^2,1], gaug=[2g,-1,-|g|^2], paug.gaug = -d.
  * loop order i(pred chunk) outer, j(gt chunk) inner; PSUM used as two
    ping-pong [128, 4x512] 4-bank tiles; ACT copies 2048-wide groups
    (amortizes the 172-cyc ScalarE overhead 4x).
  * per i, ONE 8192-wide DVE tensor_tensor col-max accumulate and ONE
    tensor_tensor_reduce (max of dh halves + fused row-max accum) —
    replaces 32 narrow DVE ops of v1.
  * no on-device partition folds: host reduces the [128, 8192] col
    partial (cheap in numpy); device tail is just the output DMA.

walrus quirk: every TPB compute instruction can carry at most ONE sync
wait. An ACT "spacer" op per i absorbs the dh-buffer WAR-on-DVE wait so
the real copies carry only their PE wait; Tile's redundant same-engine
self-waits are stripped post-trace.
"""

import numpy as np

import concourse.bass as bass
import concourse.mybir as mybir
import concourse.tile as tile
from concourse.bass_utils import run_bass_kernel_spmd

B, N, M = 4, 8192, 8192
NCORES = 8
NSH = N // 2  # pred rows per core
P = 128  # partition tile (pred rows per matmul)
FD = 512  # matmul free dim (gt cols per matmul)
NI = NSH // P  # 32 pred chunks
NJ = M // FD  # 16 gt chunks
GRP = 4  # gt chunks per PSUM group / ACT copy
NG = NJ // GRP  # 4 groups per i
GW = GRP * FD  # 2048 group width
K15 = 15  # stacked contraction dim (3 x 5 aug rows)
NEG_INF16 = -60000.0

_f32 = mybir.dt.float32
_f16 = mybir.dt.float16

_cache = {}


def _build_nc():
    nc = bass.Bass()
    # single fused input: [15, NSH pred-aug | M gt-aug] fp16 hi/lo stacks
    aT = nc.declare_dram_parameter("aT", [K15, NSH + M], _f16, isOutput=False)
    # single output: cols 0:M = colmax partial (all 128 rows valid, f16);
    # cols M:M+64 = rowmins [128,32] f32 bitcast as f16 pairs
    outall = nc.declare_dram_parameter("outall", [P, M + 64], _f16, isOutput=True)

    Alu = mybir.AluOpType
    with tile.TileContext(nc) as tc:
        with (
            tc.tile_pool(name="const", bufs=1) as cpool,
            tc.tile_pool(name="dh", bufs=2) as dhpool,
            tc.tile_pool(name="acc", bufs=1) as apool,
            tc.tile_pool(name="psd", bufs=2, space="PSUM") as psumd,
        ):
            aTs = cpool.tile([K15, NSH + M], _f16, tag="aT")
            nc.sync.dma_start(aTs[:], aT[:])
            paTs = aTs[:, :NSH]
            gaTs = aTs[:, NSH:]

            big = apool.tile([P, M + 64], _f16, tag="big")
            gmaxall = big[:, :M]
            rowmins = big[:, M:].bitcast(_f32)  # [128, 32] f32 view
            t1 = apool.tile([P, M // 2], _f16, tag="t1")  # row-tree scratch

            nc.vector.memset(gmaxall, NEG_INF16)

            for i in range(NI):
                dh = dhpool.tile([P, M], _f16, tag="dh")
                # spacer: absorbs the WAR wait against the DVE readers of
                # this buffer from iteration i-2, so the real copies below
                # carry only their PE wait (walrus 1-wait limit)
                nc.scalar.mul(dh[:, 0:M:GW], dh[:, 0:M:GW], 0.0)
                for g in range(NG):
                    pt = psumd.tile([P, GW], _f32, tag="pt")
                    for jj in range(GRP):
                        j = g * GRP + jj
                        nc.tensor.matmul(
                            pt[:, jj * FD : (jj + 1) * FD],
                            paTs[:, i * P : (i + 1) * P],
                            gaTs[:, j * FD : (j + 1) * FD],
                            start=True,
                            stop=True,
                        )
                    nc.scalar.copy(dh[:, g * GW : (g + 1) * GW], pt[:])
                # col-max accumulate over i: one 8192-wide fp16 2x op
                nc.vector.tensor_tensor(
                    out=gmaxall, in0=gmaxall, in1=dh[:], op=Alu.max
                )
                # row-max: in-place fp16 2x halving tree, then a 512-wide
                # free-dim reduce into this i's rowmins column
                nc.vector.tensor_tensor(
                    out=t1[:], in0=dh[:, : M // 2], in1=dh[:, M // 2 :], op=Alu.max
                )
                for w in (2048, 1024, 512):
                    nc.vector.tensor_tensor(
                        out=t1[:, :w], in0=t1[:, :w], in1=t1[:, w : 2 * w], op=Alu.max
                    )
                nc.vector.tensor_reduce(
                    out=rowmins[:, i : i + 1],
                    in_=t1[:, :512].rearrange("p (a f) -> p a f", a=1),
                    axis=mybir.AxisListType.X,
                    op=Alu.max,
                )

            # single output DMA: its wait on DVE transitively covers every
            # engine, so the tail drain only needs this DMA's queue sem
            nc.gpsimd.dma_start(outall[:], big[:])

    _strip_self_waits(nc)
    _slim_drain(nc)
    assert _max_tpb_waits(nc)[0] <= 1, _max_tpb_waits(nc)
    return nc


def _slim_drain(nc):
    """Reduce the kernel-tail drain to one wait (walrus 1-wait limit).

    The final output DMA waits on DVE, whose tick transitively covers all
    compute engines (ACT copies are read by DVE ops; PE matmuls are read
    by ACT copies; the aT load is awaited by the first matmul). So the
    drain only needs the output DMA's own queue semaphore.
    """
    last_q = None
    for f in nc.m.functions:
        for blk in f.blocks:
            for ins in blk.instructions:
                if type(ins).__name__ == "InstDMACopy":
                    si = ins.sync_info
                    for u in si.on_update:
                        if u.ant_name.startswith("DMASW"):
                            last_q = u.ant_name
    assert last_q is not None
    for f in nc.m.functions:
        for blk in f.blocks:
            for ins in blk.instructions:
                if type(ins).__name__ != "InstDrain":
                    continue
                si = ins.sync_info
                if si is None or len(si.on_wait) <= 1:
                    continue
                keep = [w for w in si.on_wait if w.ant_name == last_q]
                assert keep, f"drain lost its output-queue wait: {si}"
                ins.sync_info = mybir.SyncInfo(
                    on_wait=keep, on_update=list(si.on_update)
                )


_ENGINE_SEM_PREFIX = {
    mybir.EngineType.Activation: "Activation",
    mybir.EngineType.DVE: "DVE",
    mybir.EngineType.PE: "PE",
    mybir.EngineType.Pool: "Pool",
    mybir.EngineType.SP: "SP",
}


def _strip_self_waits(nc):
    """Drop a compute instruction's waits on its own engine semaphore.

    Tile models the sequencer separately from the engine and emits
    same-engine waits for buffer-slot WAW/WAR reuse; the engines complete
    in order so these are redundant, and walrus's TPB structs only encode
    one sync wait (the cross-engine wait is the essential one).
    """
    for f in nc.m.functions:
        for blk in f.blocks:
            for ins in blk.instructions:
                eng = getattr(ins, "engine", None)
                pfx = _ENGINE_SEM_PREFIX.get(eng)
                if pfx is None or type(ins).__name__ == "InstDrain":
                    continue
                si = ins.sync_info
                if si is None or not si.on_wait:
                    continue
                w2 = [w for w in si.on_wait if not w.ant_name.startswith(pfx)]
                if len(w2) != len(si.on_wait):
                    ins.sync_info = mybir.SyncInfo(
                        on_wait=w2, on_update=list(si.on_update)
                    )


def _max_tpb_waits(nc):
    """Max on_wait count over TPB compute instructions."""
    worst = (0, None)
    skip = {"InstDrain", "InstEventSemaphore", "InstISA", "InstRegisterMove"}
    for f in nc.m.functions:
        for blk in f.blocks:
            for ins in blk.instructions:
                t = type(ins).__name__
                if t in skip or t.startswith("InstDma"):
                    continue
                si = ins.sync_info
                nw = len(si.on_wait) if si else 0
                if nw > worst[0]:
                    worst = (nw, (ins.name, t, [w.ant_name for w in si.on_wait]))
    return worst


def _get_nc():
    if "nc" not in _cache:
        _cache["nc"] = _build_nc()
    return _cache["nc"]


def _augment(pred_h, gt_b):
    """pred_h [NSH,3], gt_b [M,3] -> aT [15, NSH+M] fp16 with
    sum_k aT[k,n]*aT[k,NSH+m] ~= -(squared distance n,m) to ~1e-6 abs.

    aug5(p)=[p0,p1,p2,|p|^2,1], aug5(g)=[2g0,2g1,2g2,-1,-|g|^2];
    hi/lo fp16 split, rows = [Phi;Phi;Plo] | [Ghi;Glo;Ghi]."""
    pa = np.empty((5, NSH), np.float32)
    pa[0:3] = pred_h.T
    pa[3] = (pred_h * pred_h).sum(1)
    pa[4] = 1.0
    ga = np.empty((5, M), np.float32)
    ga[0:3] = 2.0 * gt_b.T
    ga[3] = -1.0
    ga[4] = -(gt_b * gt_b).sum(1)

    phi = pa.astype(np.float16)
    plo = (pa - phi.astype(np.float32)).astype(np.float16)
    ghi = ga.astype(np.float16)
    glo = (ga - ghi.astype(np.float32)).astype(np.float16)

    aT = np.empty((K15, NSH + M), np.float16)
    aT[0:5, :NSH] = phi
    aT[5:10, :NSH] = phi
    aT[10:15, :NSH] = plo
    aT[0:5, NSH:] = ghi
    aT[5:10, NSH:] = glo
    aT[10:15, NSH:] = ghi
    return aT


def _run(pred, gt, **kwargs):
    nc = _get_nc()
    in_maps = []
    for c in range(NCORES):
        b, h = divmod(c, 2)
        in_maps.append({"aT": _augment(pred[b, h * NSH : (h + 1) * NSH], gt[b])})
    return run_bass_kernel_spmd(nc, in_maps, list(range(NCORES)), **kwargs)


def _split_out(r):
    o = r["outall"]
    colpart = o[:, :M].astype(np.float32).max(axis=0)  # [M]
    rowm = np.ascontiguousarray(o[:, M:]).view(np.float32)  # [128, NI]
    return colpart, rowm


def _combine(results):
    """results: list of 8 {'outall': [128, M+64] f16} -> scalar loss."""
    total = 0.0
    for b in range(B):
        c0, rm0 = _split_out(results[2 * b])
        c1, rm1 = _split_out(results[2 * b + 1])
        # rowmins[p, i] is pred row i*128+p -> transpose+flatten = shard order
        rm = np.concatenate([-rm0.T.reshape(-1), -rm1.T.reshape(-1)])
        cm = -np.maximum(c0, c1)
        total += rm.mean() + cm.mean()
    return np.float32(total / B)


def kernel(pred, gt):
    pred = np.ascontiguousarray(np.asarray(pred, dtype=np.float32))
    gt = np.ascontiguousarray(np.asarray(gt, dtype=np.float32))
    res = _run(pred, gt)
    return _combine(res.results)


# revision 9
# speedup vs baseline: 2.6475x; 1.1080x over previous
"""Chamfer loss kernel for Trainium2 (8 NeuronCores) — v2.

Problem: pred [4,8192,3], gt [4,8192,3] ->
  mean_b( mean_n min_m ||p_bn - g_bm||^2 + mean_m min_n ||p_bn - g_bm||^2 )

Sharding: 8 shards = (batch b in 0..3) x (half of N). Each core gets
pred half [4096,3] + full gt [8192,3] of its batch and computes
  - rowmins: min over all m for each of its 4096 pred rows
  - colpart: min over its 4096 pred rows for each of the 8192 gt points
    (as a [128, 8192] partition-partial; host folds 128->1)
Host combines (concat rows / min cols), means, final scalar.

v2 changes vs v1 (931us):
  * fp32 matmul (2 HW passes, PE-bound at 94%) replaced by a single
    fp16 matmul with K=15: each augmented vector is split hi/lo
    (hi=fp16(x), lo=fp16(x-hi)) and the three cross products
    Phi.Ghi + Phi.Glo + Plo.Ghi are stacked along the contraction dim:
    lhsT=[Phi;Phi;Plo], rhs=[Ghi;Glo;Ghi]. The dropped Plo.Glo term is
    O(1e-6) abs. PSUM accumulates fp32, so the result is fp32-accurate
    (verified 3.6e-5 rel on the full loss).
    aug: paug=[p,|p|^2,1], gaug=[2g,-1,-|g|^2], paug.gaug = -d.
  * loop order i(pred chunk) outer, j(gt chunk) inner; PSUM used as two
    ping-pong [128, 4x512] 4-bank tiles; ACT copies 2048-wide groups
    (amortizes the 172-cyc ScalarE overhead 4x).
  * per i, ONE 8192-wide DVE tensor_tensor col-max accumulate and ONE
    tensor_tensor_reduce (max of dh halves + fused row-max accum) —
    replaces 32 narrow DVE ops of v1.
  * no on-device partition folds: host reduces the [128, 8192] col
    partial (cheap in numpy); device tail is just the output DMA.

walrus quirk: every TPB compute instruction can carry at most ONE sync
wait. An ACT "spacer" op per i absorbs the dh-buffer WAR-on-DVE wait so
the real copies carry only their PE wait; Tile's redundant same-engine
self-waits are stripped post-trace.
"""

import numpy as np

import concourse.bass as bass
import concourse.mybir as mybir
import concourse.tile as tile
from concourse.bass_utils import run_bass_kernel_spmd

B, N, M = 4, 8192, 8192
NCORES = 8
NSH = N // 2  # pred rows per core
P = 128  # partition tile (pred rows per matmul)
FD = 512  # matmul free dim (gt cols per matmul)
NI = NSH // P  # 32 pred chunks
NJ = M // FD  # 16 gt chunks
GRP = 4  # gt chunks per PSUM group / ACT copy
NG = NJ // GRP  # 4 groups per i
GW = GRP * FD  # 2048 group width
K15 = 15  # stacked contraction dim (3 x 5 aug rows)
NEG_INF16 = -60000.0

_f32 = mybir.dt.float32
_f16 = mybir.dt.float16

_cache = {}


def _build_nc():
    nc = bass.Bass()
    # fused input, replicated at partition offsets 0/32/64/96 so the four
    # matmuls of a PSUM group run concurrently in distinct PE row groups
    # (tile_position row tiling): rows 32t..32t+14 all hold the same
    # [15, NSH pred-aug | M gt-aug] fp16 hi/lo stack
    aT = nc.declare_dram_parameter("aT", [P, NSH + M], _f16, isOutput=False)
    # single output: cols 0:M = colmax partial (all 128 rows valid, f16);
    # cols M:M+64 = rowmins [128,32] f32 bitcast as f16 pairs
    outall = nc.declare_dram_parameter("outall", [P, M + 64], _f16, isOutput=True)

    Alu = mybir.AluOpType
    with tile.TileContext(nc) as tc:
        with (
            tc.tile_pool(name="const", bufs=1) as cpool,
            tc.tile_pool(name="dh", bufs=2) as dhpool,
            tc.tile_pool(name="acc", bufs=1) as apool,
            tc.tile_pool(name="psd", bufs=2, space="PSUM") as psumd,
        ):
            aTs = cpool.tile([P, NSH + M], _f16, tag="aT")
            nc.sync.dma_start(aTs[:], aT[:])

            big = apool.tile([P, M + 64], _f16, tag="big")
            gmaxall = big[:, :M]
            rowmins = big[:, M:].bitcast(_f32)  # [128, 32] f32 view
            t1 = apool.tile([P, M // 2], _f16, tag="t1")  # row-tree scratch
            junk = apool.tile([P, 2048], _f16, tag="junk")

            nc.vector.memset(gmaxall, NEG_INF16)

            for i in range(NI):
                dh = dhpool.tile([P, M], _f16, tag="dh")
                # spacer: absorbs the WAR wait against the DVE readers of
                # this buffer from iteration i-2, so the real copies below
                # carry only their PE wait (walrus 1-wait limit)
                nc.scalar.mul(dh[:, 0:M:GW], dh[:, 0:M:GW], 0.0)
                for g in range(NG):
                    pt = psumd.tile([P, GW], _f32, tag="pt")
                    for jj in range(GRP):
                        j = g * GRP + jj
                        bp = 32 * jj
                        nc.tensor.matmul(
                            pt[:, jj * FD : (jj + 1) * FD],
                            aTs[bp : bp + K15, i * P : (i + 1) * P],
                            aTs[bp : bp + K15, NSH + j * FD : NSH + (j + 1) * FD],
                            start=True,
                            stop=True,
                            tile_position=(bp, 0),
                        )
                    nc.scalar.copy(dh[:, g * GW : (g + 1) * GW], pt[:])
                # col-max accumulate over i: one 8192-wide fp16 2x op
                nc.vector.tensor_tensor(
                    out=gmaxall, in0=gmaxall, in1=dh[:], op=Alu.max
                )
                # row-max: two fp16 2x halving levels, then a fused
                # copy+max-accum over the 1024-wide remainder
                nc.vector.tensor_tensor(
                    out=t1[:], in0=dh[:, : M // 2], in1=dh[:, M // 2 :], op=Alu.max
                )
                nc.vector.tensor_tensor(
                    out=t1[:, :2048], in0=t1[:, :2048], in1=t1[:, 2048:4096], op=Alu.max
                )
                nc.vector.tensor_scalar(
                    out=junk[:],
                    in0=t1[:, :2048],
                    scalar1=0.0,
                    scalar2=None,
                    op0=Alu.add,
                    op1=Alu.max,
                    accum_out=rowmins[:, i : i + 1],
                )

            # single output DMA: its wait on DVE transitively covers every
            # engine, so the tail drain only needs this DMA's queue sem
            nc.gpsimd.dma_start(outall[:], big[:])

    _strip_self_waits(nc)
    _slim_drain(nc)
    assert _max_tpb_waits(nc)[0] <= 1, _max_tpb_waits(nc)
    return nc


def _slim_drain(nc):
    """Reduce the kernel-tail drain to one wait (walrus 1-wait limit).

    The final output DMA waits on DVE, whose tick transitively covers all
    compute engines (ACT copies are read by DVE ops; PE matmuls are read
    by ACT copies; the aT load is awaited by the first matmul). So the
    drain only needs the output DMA's own queue semaphore.
    """
    last_q = None
    for f in nc.m.functions:
        for blk in f.blocks:
            for ins in blk.instructions:
                if type(ins).__name__ == "InstDMACopy":
                    si = ins.sync_info
                    for u in si.on_update:
                        if u.ant_name.startswith("DMASW"):
                            last_q = u.ant_name
    assert last_q is not None
    for f in nc.m.functions:
        for blk in f.blocks:
            for ins in blk.instructions:
                if type(ins).__name__ != "InstDrain":
                    continue
                si = ins.sync_info
                if si is None or len(si.on_wait) <= 1:
                    continue
                keep = [w for w in si.on_wait if w.ant_name == last_q]
                assert keep, f"drain lost its output-queue wait: {si}"
                ins.sync_info = mybir.SyncInfo(
                    on_wait=keep, on_update=list(si.on_update)
                )


_ENGINE_SEM_PREFIX = {
    mybir.EngineType.Activation: "Activation",
    mybir.EngineType.DVE: "DVE",
    mybir.EngineType.PE: "PE",
    mybir.EngineType.Pool: "Pool",
    mybir.EngineType.SP: "SP",
}


def _strip_self_waits(nc):
    """Drop a compute instruction's waits on its own engine semaphore.

    Tile models the sequencer separately from the engine and emits
    same-engine waits for buffer-slot WAW/WAR reuse; the engines complete
    in order so these are redundant, and walrus's TPB structs only encode
    one sync wait (the cross-engine wait is the essential one).
    """
    for f in nc.m.functions:
        for blk in f.blocks:
            for ins in blk.instructions:
                eng = getattr(ins, "engine", None)
                pfx = _ENGINE_SEM_PREFIX.get(eng)
                if pfx is None or type(ins).__name__ == "InstDrain":
                    continue
                si = ins.sync_info
                if si is None or not si.on_wait:
                    continue
                w2 = [w for w in si.on_wait if not w.ant_name.startswith(pfx)]
                if len(w2) != len(si.on_wait):
                    ins.sync_info = mybir.SyncInfo(
                        on_wait=w2, on_update=list(si.on_update)
                    )


def _max_tpb_waits(nc):
    """Max on_wait count over TPB compute instructions."""
    worst = (0, None)
    skip = {"InstDrain", "InstEventSemaphore", "InstISA", "InstRegisterMove"}
    for f in nc.m.functions:
        for blk in f.blocks:
            for ins in blk.instructions:
                t = type(ins).__name__
                if t in skip or t.startswith("InstDma"):
                    continue
                si = ins.sync_info
                nw = len(si.on_wait) if si else 0
                if nw > worst[0]:
                    worst = (nw, (ins.name, t, [w.ant_name for w in si.on_wait]))
    return worst


def _get_nc():
    if "nc" not in _cache:
        _cache["nc"] = _build_nc()
    return _cache["nc"]


def _augment(pred_h, gt_b):
    """pred_h [NSH,3], gt_b [M,3] -> aT [15, NSH+M] fp16 with
    sum_k aT[k,n]*aT[k,NSH+m] ~= -(squared distance n,m) to ~1e-6 abs.

    aug5(p)=[p0,p1,p2,|p|^2,1], aug5(g)=[2g0,2g1,2g2,-1,-|g|^2];
    hi/lo fp16 split, rows = [Phi;Phi;Plo] | [Ghi;Glo;Ghi]."""
    pa = np.empty((5, NSH), np.float32)
    pa[0:3] = pred_h.T
    pa[3] = (pred_h * pred_h).sum(1)
    pa[4] = 1.0
    ga = np.empty((5, M), np.float32)
    ga[0:3] = 2.0 * gt_b.T
    ga[3] = -1.0
    ga[4] = -(gt_b * gt_b).sum(1)

    phi = pa.astype(np.float16)
    plo = (pa - phi.astype(np.float32)).astype(np.float16)
    ghi = ga.astype(np.float16)
    glo = (ga - ghi.astype(np.float32)).astype(np.float16)

    aT = np.zeros((P, NSH + M), np.float16)
    for t in range(4):  # replicas at partition offsets 0/32/64/96
        bp = 32 * t
        aT[bp : bp + 5, :NSH] = phi
        aT[bp + 5 : bp + 10, :NSH] = phi
        aT[bp + 10 : bp + 15, :NSH] = plo
        aT[bp : bp + 5, NSH:] = ghi
        aT[bp + 5 : bp + 10, NSH:] = glo
        aT[bp + 10 : bp + 15, NSH:] = ghi
    return aT


def _run(pred, gt, **kwargs):
    nc = _get_nc()
    in_maps = []
    for c in range(NCORES):
        b, h = divmod(c, 2)
        in_maps.append({"aT": _augment(pred[b, h * NSH : (h + 1) * NSH], gt[b])})
    return run_bass_kernel_spmd(nc, in_maps, list(range(NCORES)), **kwargs)


def _split_out(r):
    o = r["outall"]
    colpart = o[:, :M].astype(np.float32).max(axis=0)  # [M]
    rowm = np.ascontiguousarray(o[:, M:]).view(np.float32)  # [128, NI]
    return colpart, rowm


def _combine(results):
    """results: list of 8 {'outall': [128, M+64] f16} -> scalar loss."""
    total = 0.0
    for b in range(B):
        c0, rm0 = _split_out(results[2 * b])
        c1, rm1 = _split_out(results[2 * b + 1])
        # rowmins[p, i] is pred row i*128+p -> transpose+flatten = shard order
        rm = np.concatenate([-rm0.T.reshape(-1), -rm1.T.reshape(-1)])
        cm = -np.maximum(c0, c1)
        total += rm.mean() + cm.mean()
    return np.float32(total / B)


def kernel(pred, gt):
    pred = np.ascontiguousarray(np.asarray(pred, dtype=np.float32))
    gt = np.ascontiguousarray(np.asarray(gt, dtype=np.float32))
    res = _run(pred, gt)
    return _combine(res.results)


# revision 18
# speedup vs baseline: 2.8188x; 1.0647x over previous
"""Chamfer loss kernel for Trainium2 (8 NeuronCores) — v2.

Problem: pred [4,8192,3], gt [4,8192,3] ->
  mean_b( mean_n min_m ||p_bn - g_bm||^2 + mean_m min_n ||p_bn - g_bm||^2 )

Sharding: 8 shards = (batch b in 0..3) x (half of N). Each core gets
pred half [4096,3] + full gt [8192,3] of its batch and computes
  - rowmins: min over all m for each of its 4096 pred rows
  - colpart: min over its 4096 pred rows for each of the 8192 gt points
    (as a [128, 8192] partition-partial; host folds 128->1)
Host combines (concat rows / min cols), means, final scalar.

Changes vs the 931us baseline (now ~335us):
  * fp32 matmul (2 HW passes each, PE-bound at 94%) replaced by a single
    fp16 matmul with K=15: each augmented vector is split hi/lo
    (hi=fp16(x), lo=fp16(x-hi)) and the three cross products
    Phi.Ghi + Phi.Glo + Plo.Ghi are stacked along the contraction dim:
    lhsT=[Phi;Phi;Plo], rhs=[Ghi;Glo;Ghi]. The dropped Plo.Glo term is
    O(1e-6) abs. PSUM accumulates fp32, so the result is fp32-accurate
    (verified 3.6e-5 rel on the full loss).
    aug: paug=[p,|p|^2,1], gaug=[2g,-1,-|g|^2], paug.gaug = -d.
  * the 4 matmuls of a PSUM group run concurrently in distinct PE
    32-row groups (tile_position row tiling; K=15 <= 32) — the input is
    replicated at partition offsets 0/32/64/96 to feed the groups.
  * loop order i(pred chunk) outer, j(gt chunk) inner; PSUM used as two
    ping-pong [128, 4x512] 4-bank tiles; ACT copies 2048-wide groups
    (amortizes the 172-cyc ScalarE overhead 4x).
  * per i, ONE 8192-wide DVE tensor_tensor col-max accumulate (fp16 2x)
    plus a 4-level in-place halving tree + 512-wide fused max-accum for
    the row mins — 6 wide DVE ops replace 32 narrow ones.
  * no on-device partition folds: host reduces the [128, 8192] col
    partial (cheap in numpy); device tail is just the output DMA.
  Engine busy at final state: DVE ~95% (bottleneck), ACT ~77%, PE ~60%
  effective (4-way row-group overlap).

walrus quirk: every TPB compute instruction can carry at most ONE sync
wait. An ACT "spacer" op per i absorbs the dh-buffer WAR-on-DVE wait so
the real copies carry only their PE wait; Tile's redundant same-engine
self-waits are stripped post-trace.
"""

import numpy as np

import concourse.bass as bass
import concourse.mybir as mybir
import concourse.tile as tile
from concourse.bass_utils import run_bass_kernel_spmd

B, N, M = 4, 8192, 8192
NCORES = 8
NSH = N // 2  # pred rows per core
P = 128  # partition tile (pred rows per matmul)
FD = 512  # matmul free dim (gt cols per matmul)
NI = NSH // P  # 32 pred chunks
NJ = M // FD  # 16 gt chunks
GRP = 4  # gt chunks per PSUM group / ACT copy
NG = NJ // GRP  # 4 groups per i
GW = GRP * FD  # 2048 group width
K15 = 15  # stacked contraction dim (3 x 5 aug rows)
NEG_INF16 = -60000.0

_f32 = mybir.dt.float32
_f16 = mybir.dt.float16

_cache = {}


def _build_nc():
    nc = bass.Bass()
    # fused input, replicated at partition offsets 0/32/64/96 so the four
    # matmuls of a PSUM group run concurrently in distinct PE row groups
    # (tile_position row tiling): rows 32t..32t+14 all hold the same
    # [15, NSH pred-aug | M gt-aug] fp16 hi/lo stack
    aT = nc.declare_dram_parameter("aT", [P, NSH + M], _f16, isOutput=False)
    # single output: cols 0:M = colmax partial (all 128 rows valid, f16);
    # cols M:M+64 = rowmins [128,32] f32 bitcast as f16 pairs
    outall = nc.declare_dram_parameter("outall", [P, M + 64], _f16, isOutput=True)

    Alu = mybir.AluOpType
    with tile.TileContext(nc) as tc:
        with (
            tc.tile_pool(name="const", bufs=1) as cpool,
            tc.tile_pool(name="dh", bufs=2) as dhpool,
            tc.tile_pool(name="acc", bufs=1) as apool,
            tc.tile_pool(name="psd", bufs=2, space="PSUM") as psumd,
        ):
            aTs = cpool.tile([P, NSH + M], _f16, tag="aT")
            nc.sync.dma_start(aTs[:], aT[:])

            big = apool.tile([P, M + 64], _f16, tag="big")
            gmaxall = big[:, :M]
            rowmins = big[:, M:].bitcast(_f32)  # [128, 32] f32 view
            t1 = apool.tile([P, M // 2], _f16, tag="t1")  # row-tree scratch
            junk = apool.tile([P, 512], _f16, tag="junk")

            for i in range(NI):
                dh = dhpool.tile([P, M], _f16, tag="dh")
                # spacer: absorbs the WAR wait against the DVE readers of
                # this buffer from iteration i-2, so the real copies below
                # carry only their PE wait (walrus 1-wait limit)
                nc.scalar.mul(dh[:, 0:1], dh[:, 0:1], 0.0)
                for g in range(NG):
                    pt = psumd.tile([P, GW], _f32, tag="pt")
                    for jj in range(GRP):
                        j = g * GRP + jj
                        bp = 32 * jj
                        nc.tensor.matmul(
                            pt[:, jj * FD : (jj + 1) * FD],
                            aTs[bp : bp + K15, i * P : (i + 1) * P],
                            aTs[bp : bp + K15, NSH + j * FD : NSH + (j + 1) * FD],
                            start=True,
                            stop=True,
                            tile_position=(bp, 0),
                        )
                    nc.scalar.copy(dh[:, g * GW : (g + 1) * GW], pt[:])
                # col-max accumulate over i: one 8192-wide fp16 2x op.
                # i=0 initializes via a plain copy (4x mode) — replaces
                # both the gmaxall memset and the first RMW accumulate.
                if i == 0:
                    nc.vector.tensor_copy(gmaxall, dh[:])
                else:
                    nc.vector.tensor_tensor(
                        out=gmaxall, in0=gmaxall, in1=dh[:], op=Alu.max
                    )
                # row-max: two fp16 2x halving levels, then a fused
                # copy+max-accum over the 1024-wide remainder
                nc.vector.tensor_tensor(
                    out=t1[:], in0=dh[:, : M // 2], in1=dh[:, M // 2 :], op=Alu.max
                )
                for w in (2048, 1024, 512):
                    nc.vector.tensor_tensor(
                        out=t1[:, :w], in0=t1[:, :w], in1=t1[:, w : 2 * w], op=Alu.max
                    )
                nc.vector.tensor_scalar(
                    out=junk[:],
                    in0=t1[:, :512],
                    scalar1=0.0,
                    scalar2=None,
                    op0=Alu.add,
                    op1=Alu.max,
                    accum_out=rowmins[:, i : i + 1],
                )

            # single output DMA: its wait on DVE transitively covers every
            # engine, so the tail drain only needs this DMA's queue sem
            nc.gpsimd.dma_start(outall[:], big[:])

    _strip_self_waits(nc)
    _slim_drain(nc)
    assert _max_tpb_waits(nc)[0] <= 1, _max_tpb_waits(nc)
    return nc


def _slim_drain(nc):
    """Reduce the kernel-tail drain to one wait (walrus 1-wait limit).

    The final output DMA waits on DVE, whose tick transitively covers all
    compute engines (ACT copies are read by DVE ops; PE matmuls are read
    by ACT copies; the aT load is awaited by the first matmul). So the
    drain only needs the output DMA's own queue semaphore.
    """
    last_q = None
    for f in nc.m.functions:
        for blk in f.blocks:
            for ins in blk.instructions:
                if type(ins).__name__ == "InstDMACopy":
                    si = ins.sync_info
                    for u in si.on_update:
                        if u.ant_name.startswith("DMASW"):
                            last_q = u.ant_name
    assert last_q is not None
    for f in nc.m.functions:
        for blk in f.blocks:
            for ins in blk.instructions:
                if type(ins).__name__ != "InstDrain":
                    continue
                si = ins.sync_info
                if si is None or len(si.on_wait) <= 1:
                    continue
                keep = [w for w in si.on_wait if w.ant_name == last_q]
                assert keep, f"drain lost its output-queue wait: {si}"
                ins.sync_info = mybir.SyncInfo(
                    on_wait=keep, on_update=list(si.on_update)
                )


_ENGINE_SEM_PREFIX = {
    mybir.EngineType.Activation: "Activation",
    mybir.EngineType.DVE: "DVE",
    mybir.EngineType.PE: "PE",
    mybir.EngineType.Pool: "Pool",
    mybir.EngineType.SP: "SP",
}


def _strip_self_waits(nc):
    """Drop a compute instruction's waits on its own engine semaphore.

    Tile models the sequencer separately from the engine and emits
    same-engine waits for buffer-slot WAW/WAR reuse; the engines complete
    in order so these are redundant, and walrus's TPB structs only encode
    one sync wait (the cross-engine wait is the essential one).
    """
    for f in nc.m.functions:
        for blk in f.blocks:
            for ins in blk.instructions:
                eng = getattr(ins, "engine", None)
                pfx = _ENGINE_SEM_PREFIX.get(eng)
                if pfx is None or type(ins).__name__ == "InstDrain":
                    continue
                si = ins.sync_info
                if si is None or not si.on_wait:
                    continue
                w2 = [w for w in si.on_wait if not w.ant_name.startswith(pfx)]
                if len(w2) != len(si.on_wait):
                    ins.sync_info = mybir.SyncInfo(
                        on_wait=w2, on_update=list(si.on_update)
                    )


def _max_tpb_waits(nc):
    """Max on_wait count over TPB compute instructions."""
    worst = (0, None)
    skip = {"InstDrain", "InstEventSemaphore", "InstISA", "InstRegisterMove"}
    for f in nc.m.functions:
        for blk in f.blocks:
            for ins in blk.instructions:
                t = type(ins).__name__
                if t in skip or t.startswith("InstDma"):
                    continue
                si = ins.sync_info
                nw = len(si.on_wait) if si else 0
                if nw > worst[0]:
                    worst = (nw, (ins.name, t, [w.ant_name for w in si.on_wait]))
    return worst


def _get_nc():
    if "nc" not in _cache:
        _cache["nc"] = _build_nc()
    return _cache["nc"]


def _augment(pred_h, gt_b):
    """pred_h [NSH,3], gt_b [M,3] -> aT [15, NSH+M] fp16 with
    sum_k aT[k,n]*aT[k,NSH+m] ~= -(squared distance n,m) to ~1e-6 abs.

    aug5(p)=[p0,p1,p2,|p|^2,1], aug5(g)=[2g0,2g1,2g2,-1,-|g|^2];
    hi/lo fp16 split, rows = [Phi;Phi;Plo] | [Ghi;Glo;Ghi]."""
    pa = np.empty((5, NSH), np.float32)
    pa[0:3] = pred_h.T
    pa[3] = (pred_h * pred_h).sum(1)
    pa[4] = 1.0
    ga = np.empty((5, M), np.float32)
    ga[0:3] = 2.0 * gt_b.T
    ga[3] = -1.0
    ga[4] = -(gt_b * gt_b).sum(1)

    phi = pa.astype(np.float16)
    plo = (pa - phi.astype(np.float32)).astype(np.float16)
    ghi = ga.astype(np.float16)
    glo = (ga - ghi.astype(np.float32)).astype(np.float16)

    aT = np.zeros((P, NSH + M), np.float16)
    for t in range(4):  # replicas at partition offsets 0/32/64/96
        bp = 32 * t
        aT[bp : bp + 5, :NSH] = phi
        aT[bp + 5 : bp + 10, :NSH] = phi
        aT[bp + 10 : bp + 15, :NSH] = plo
        aT[bp : bp + 5, NSH:] = ghi
        aT[bp + 5 : bp + 10, NSH:] = glo
        aT[bp + 10 : bp + 15, NSH:] = ghi
    return aT


def _run(pred, gt, **kwargs):
    nc = _get_nc()
    in_maps = []
    for c in range(NCORES):
        b, h = divmod(c, 2)
        in_maps.append({"aT": _augment(pred[b, h * NSH : (h + 1) * NSH], gt[b])})
    return run_bass_kernel_spmd(nc, in_maps, list(range(NCORES)), **kwargs)


def _split_out(r):
    o = r["outall"]
    colpart = o[:, :M].astype(np.float32).max(axis=0)  # [M]
    rowm = np.ascontiguousarray(o[:, M:]).view(np.float32)  # [128, NI]
    return colpart, rowm


def _combine(results):
    """results: list of 8 {'outall': [128, M+64] f16} -> scalar loss."""
    total = 0.0
    for b in range(B):
        c0, rm0 = _split_out(results[2 * b])
        c1, rm1 = _split_out(results[2 * b + 1])
        # rowmins[p, i] is pred row i*128+p -> transpose+flatten = shard order
        rm = np.concatenate([-rm0.T.reshape(-1), -rm1.T.reshape(-1)])
        cm = -np.maximum(c0, c1)
        total += rm.mean() + cm.mean()
    return np.float32(total / B)


def kernel(pred, gt):
    pred = np.ascontiguousarray(np.asarray(pred, dtype=np.float32))
    gt = np.ascontiguousarray(np.asarray(gt, dtype=np.float32))
    res = _run(pred, gt)
    return _combine(res.results)
